# revision 6
# baseline (speedup 1.0000x reference)
"""2-layer GCN (gnn_message_passing) on 8 Trainium2 NeuronCores.

Strategy (v2, source-sharded + ReduceScatter):
  Edges are assigned to the core owning the SOURCE node (col). Each core:
    support1 = X_local @ W1 -> local table tab1 (12544 x 128 fp16, DRAM)
    scatter: for every global dest cell (256 rows = 2 blocks), gather
      sup rows by local col idx (dma_gather, int16 idx), one-hot matmul
      (transposed: psum = gt.T @ onehot -> (hid, dest)) into per-block psum,
      emit fp16 partial blocks into part1 (8, 128, 12544).
    ReduceScatter(add) -> rs1 (128, 12544): this core's h1 shard, transposed.
    h1 = relu(rs1); tab2 = h1.T @ W2 per block (no transpose needed).
    Same scatter for layer 2 (64-wide table rows, 128B gather elems),
    ReduceScatter -> rs2 (64, 12544), PE-transpose -> out f32.
  One-hot built with a parity trick (cell rows valued 0..255) so chunks can
  span the two blocks of a cell; rowloc stored duplicated x2 so the
  is_equal TensorTensor qualifies for the DVE 2x_1p fast mode.
"""
import sys
sys.path.insert(0, "/opt/trn_rl_repo")

import numpy as np
from contextlib import ExitStack

import concourse.bass as bass
import concourse.bacc as bacc
import concourse.tile as tile
from concourse import bass_utils
from concourse import mybir
from concourse.library_config import mlp

PADVAL = 300.0
GMAX = 8          # 128-idx chunks per dma_gather call
SCRATCH = 16384   # dynamic_dma_scratch_size (bytes; /16 = ring descs)
CELLR = 256       # dest rows per cell (2 blocks of 128)


class Config:
    def __init__(self, n=100000, in_dim=256, hid=128, out_dim=64, ncore=8):
        self.N = n
        self.IN = in_dim
        self.HID = hid
        self.OUT = out_dim
        self.NCORE = ncore
        self.NPC = n // ncore
        assert self.NPC * ncore == n
        self.NB = (self.NPC + 127) // 128
        self.NPP = self.NB * 128
        self.NT = ncore * self.NPP
        self.NCELL = self.NT // CELLR
        self.KT = in_dim // 128


CFG = Config()


def prep_edges(cfg, edge_index):
    """Bucket edges by (src core, dest cell); within a cell order by dest
    block (A = even block, B = odd). Chunk counts per cell are uniform
    across cores (max). Returns per-core colidx (int16, 16-wrapped),
    rowloc2 (fp16, duplicated x2), and compile-time cell metadata."""
    c = cfg
    row = np.asarray(edge_index[0], dtype=np.int64)
    col = np.asarray(edge_index[1], dtype=np.int64)
    src = col // c.NPC
    tloc = (col - src * c.NPC).astype(np.int16)
    rT = (row // c.NPC) * c.NPP + (row % c.NPC)
    cell = rT // CELLR
    par = (rT // 128) % 2
    rib = (rT % CELLR).astype(np.float16)  # 0..255, parity-coded row in cell

    key = (src * c.NCELL + cell) * 2 + par
    order = np.argsort(key, kind="stable")
    tloc_s = tloc[order]
    rib_s = rib[order]
    ncel2 = c.NCORE * c.NCELL * 2
    counts2 = np.bincount(key[order], minlength=ncel2).reshape(
        c.NCORE, c.NCELL, 2)
    cntA = counts2[:, :, 0]
    cntAB = counts2.sum(axis=2)
    starts = np.concatenate([[0], np.cumsum(counts2.reshape(-1))])

    m_cell = np.maximum(-(-cntAB.max(axis=0) // 128), 1)       # chunks/cell
    chunksA = np.maximum(-(-cntA.max(axis=0) // 128), 1)
    bstart = np.minimum(cntA.min(axis=0) // 128, m_cell - 1)
    cbase = np.concatenate([[0], np.cumsum(m_cell)])
    nch = int(cbase[-1])

    colidx = np.zeros((c.NCORE, 128, nch * 8), np.int16)
    rowloc2 = np.full((c.NCORE, 128, nch * 2), PADVAL, np.float16)

    for cc in range(c.NCORE):
        for ce in range(c.NCELL):
            nk = int(m_cell[ce])
            cap = nk * 128
            kiA = (cc * c.NCELL + ce) * 2
            sA, sB, eB = starts[kiA], starts[kiA + 1], starts[kiA + 2]
            cnt = int(eB - sA)
            tl = np.zeros(cap, np.int16)
            tl[:cnt] = tloc_s[sA:eB]
            rb = np.full(cap, PADVAL, np.float16)
            rb[:cnt] = rib_s[sA:eB]
            qo = int(cbase[ce])
            colidx[cc][:, qo * 8:(qo + nk) * 8] = np.tile(
                tl.reshape(-1, 16).T, (8, 1))
            r2 = rb.reshape(nk, 128).T  # (128, nk)
            rowloc2[cc][:, qo * 2:(qo + nk) * 2] = np.repeat(r2, 2, axis=1)
    meta = dict(m_cell=m_cell, chunksA=chunksA, bstart=bstart,
                cbase=cbase, nch=nch)
    return colidx, rowloc2, meta


def _dma_gather_raw(gp, out_ap, in_ap, idxs_ap, num_idxs, num_idxs_reg,
                    elem_size, elem_step, queue_num):
    """dma_gather (non-transpose, DRAM source) without the %256 payload
    assert — the ucode only requires 256B multiples for transpose mode;
    the source ROW STRIDE must still be a 256B multiple."""
    gp._assert_queue_num(queue_num)
    assert idxs_ap.dtype == mybir.dt.int16
    assert in_ap.dtype == out_ap.dtype
    elem_size_bytes = elem_size * mybir.dt.size(in_ap.dtype)
    assert elem_size_bytes > 0 and elem_size_bytes % 128 == 0
    assert in_ap.space == bass.MemorySpace.DRAM
    assert idxs_ap.space == bass.MemorySpace.SBUF
    assert out_ap.space == bass.MemorySpace.SBUF
    assert in_ap.ap[-1][1] == out_ap.ap[-1][1] == elem_size
    assert out_ap.ap[0][1] * out_ap.ap[1][1] == ((num_idxs + 127) // 128) * 128
    assert in_ap.ap[0][0] == elem_step
    stride_bytes = elem_step * mybir.dt.size(in_ap.dtype)
    stride_bytes_256 = stride_bytes // 256
    assert stride_bytes_256 * 256 == stride_bytes and stride_bytes_256 < 256
    _in_ap = gp.lower_ap_dma(in_ap, for_custom_bir_dma=True)
    _idxs_ap = gp.lower_ap(idxs_ap)
    _out_ap = gp.lower_ap(out_ap)
    return gp.add_instruction(
        mybir.InstDMAGatherAnt(
            name=gp.bass.get_next_instruction_name(),
            ins=[*_in_ap, _idxs_ap,
                 gp.lower_val_access(gp.to_reg(num_idxs_reg))],
            outs=[_out_ap],
            transpose=False, num_idxs=num_idxs, elem_size=elem_size,
            stride_bytes_256=stride_bytes_256, gen_mode=0,
            single_packet=True, queue_num=queue_num,
            sbuf_tokens_per_rank=0, sbuf_free_dim_per_rank=0,
            sbuf_free_dim_pad_per_rank=0, sbuf_byte_offset=0))


def build(cfg, meta):
    c = cfg
    m_cell, chunksA, bstart, cbase, nch = (
        meta["m_cell"], meta["chunksA"], meta["bstart"], meta["cbase"],
        meta["nch"])
    nc = bacc.Bacc(None, target_bir_lowering=False, debug=False,
                   num_devices=c.NCORE, name="gcn2", num_swdge_queues=4,
                   dynamic_dma_scratch_size=SCRATCH)
    f16, f32, i16 = mybir.dt.float16, mybir.dt.float32, mybir.dt.int16
    iseq = mybir.AluOpType.is_equal

    xT = nc.dram_tensor("xT", (c.IN, c.NPP), f16, kind="ExternalInput")
    w1 = nc.dram_tensor("w1", (c.IN, c.HID), f16, kind="ExternalInput")
    w2 = nc.dram_tensor("w2", (c.HID, c.OUT), f16, kind="ExternalInput")
    ident = nc.dram_tensor("ident", (128, 128), f16, kind="ExternalInput")
    iotaAB = nc.dram_tensor("iotaAB", (128, 256), f16, kind="ExternalInput")
    colidx = nc.dram_tensor("colidx", (128, nch * 8), i16,
                            kind="ExternalInput")
    rowloc2 = nc.dram_tensor("rowloc2", (128, nch * 2), f16,
                             kind="ExternalInput")
    out = nc.dram_tensor("out", (c.NPC, c.OUT), f32, kind="ExternalOutput")

    tab1 = nc.dram_tensor("tab1", (c.NPP, c.HID), f16, kind="Internal")
    part1 = nc.dram_tensor("part1", (c.NCORE, c.HID, c.NPP), f16,
                           kind="Internal")
    rs1 = nc.dram_tensor("rs1", (c.HID, c.NPP), f16, kind="Internal")
    tab2 = nc.dram_tensor("tab2", (c.NPP, 128), f16, kind="Internal")
    part2 = nc.dram_tensor("part2", (c.NCORE, c.OUT, c.NPP), f16,
                           kind="Internal")
    rs2 = nc.dram_tensor("rs2", (c.OUT, c.NPP), f16, kind="Internal")

    groups = [list(range(c.NCORE))]
    cells_per_k = c.NPP // CELLR  # dest cells per core shard

    with ExitStack() as ctx:
        tc = ctx.enter_context(tile.TileContext(nc))
        nc.gpsimd.load_library(mlp)
        cpool = ctx.enter_context(tc.tile_pool(name="const", bufs=1))
        ident_sb = cpool.tile((128, 128), f16, tag="ident")
        nc.sync.dma_start(ident_sb[:], ident[:])
        iota_sb = cpool.tile((128, 256), f16, tag="iota")
        nc.sync.dma_start(iota_sb[:], iotaAB[:])
        w2_sb = cpool.tile((c.HID, c.OUT), f16, tag="w2")
        nc.sync.dma_start(w2_sb[:], w2[:])
        rl2_sb = cpool.tile((128, nch * 2), f16, tag="rl2")
        nc.sync.dma_start(rl2_sb[:], rowloc2[:])

        # Phase A: tab1 = X @ W1 per block
        with tc.tile_pool(name="pa", bufs=1) as pa, \
             tc.tile_pool(name="pas", bufs=3) as pas, \
             tc.tile_pool(name="psa", bufs=2, space="PSUM") as psa:
            xk = []
            w1k = []
            for k in range(c.KT):
                t = pa.tile((128, c.NPP), f16, tag=f"x{k}", name=f"xk{k}")
                nc.sync.dma_start(t[:], xT.ap()[k * 128:(k + 1) * 128, :])
                xk.append(t)
                t = pa.tile((128, c.HID), f16, tag=f"w{k}", name=f"w1k{k}")
                nc.sync.dma_start(t[:], w1.ap()[k * 128:(k + 1) * 128, :])
                w1k.append(t)
            for b in range(c.NB):
                ps = psa.tile((128, c.HID), f32, space="PSUM")
                for k in range(c.KT):
                    nc.tensor.matmul(ps[:], xk[k][:, b * 128:(b + 1) * 128],
                                     w1k[k][:], start=(k == 0),
                                     stop=(k == c.KT - 1))
                s1 = pas.tile((128, c.HID), f16)
                nc.scalar.activation(s1[:], ps[:],
                                     mybir.ActivationFunctionType.Copy)
                nc.sync.dma_start(tab1.ap()[b * 128:(b + 1) * 128, :], s1[:])

        def scatter_layer(tab_ap, elem, estep, W, part):
            """Gather+scatter all dest cells. Table rows `elem` fp16 payload
            at stride `estep`; W = feature width; psum blocks transposed
            (W, 128); partial cells (W, 256) -> part (NCORE, W, NPP)."""
            gq = [0]
            call_tile = {}

            with tc.tile_pool(name="poh", bufs=4) as poh, \
                 tc.tile_pool(name="pg", bufs=6) as pg, \
                 tc.tile_pool(name="pix", bufs=6) as pix, \
                 tc.tile_pool(name="pem", bufs=3) as pem, \
                 tc.tile_pool(name="pso", bufs=3, space="PSUM") as pso:

                def ensure_call(q):
                    if q in call_tile:
                        return call_tile[q]
                    sn = min(GMAX, nch - q * GMAX)
                    it = pix.tile((128, sn * 8), i16)
                    nc.sync.dma_start(
                        it[:], colidx.ap()[:, q * GMAX * 8:(q * GMAX + sn) * 8])
                    gt = pg.tile((128, sn, elem), f16)
                    _dma_gather_raw(
                        nc.gpsimd, gt[:], tab_ap, it[:], sn * 128, sn * 128,
                        elem, estep, gq[0] % 4)
                    gq[0] += 1
                    call_tile[q] = (gt, sn)
                    return call_tile[q]

                for ce in range(c.NCELL):
                    cb = int(cbase[ce])
                    m = int(m_cell[ce])
                    cA = int(chunksA[ce])
                    bs = int(bstart[ce])
                    ncols = cA + (m - bs)
                    oh = poh.tile((128, ncols, 128), f16)
                    # one-hot: A-cols vs iota 0..127, B-cols vs 128..255
                    for (c0, c1, pslice, j0) in (
                            (0, cA, 0, 0), (cA, ncols, 1, bs)):
                        nco = c1 - c0
                        ox = oh[:, c0:c1, :]
                        ov = bass.AP(ox.tensor, ox.offset,
                                     [ox.ap[0], [128, nco], [2, 64], [1, 2]])
                        ix = iota_sb[:, pslice * 128:(pslice + 1) * 128]
                        iv = bass.AP(ix.tensor, ix.offset,
                                     [ix.ap[0], [0, nco], [2, 64], [1, 2]])
                        rx = rl2_sb[:, 2 * (cb + j0):2 * (cb + j0) + 2 * nco]
                        rv = bass.AP(rx.tensor, rx.offset,
                                     [rx.ap[0], [2, nco], [0, 64], [1, 2]])
                        nc.vector.tensor_tensor(out=ov, in0=iv, in1=rv,
                                                op=iseq)
                    psA = pso.tile((W, 128), f32, space="PSUM", tag="psA",
                                   name="psA")
                    psB = pso.tile((W, 128), f32, space="PSUM", tag="psB",
                                   name="psB")
                    for j in range(m):
                        gi = cb + j
                        gt, sn = ensure_call(gi // GMAX)
                        sl = gt[:, gi % GMAX, 0:W]
                        if j < cA:
                            nc.tensor.matmul(psA[:], sl, oh[:, j, :],
                                             start=(j == 0),
                                             stop=(j == cA - 1))
                        if j >= bs:
                            nc.tensor.matmul(psB[:], sl,
                                             oh[:, cA + (j - bs), :],
                                             start=(j == bs),
                                             stop=(j == m - 1))
                    em = pem.tile((W, CELLR), f16)
                    nc.scalar.activation(em[:, 0:128], psA[:],
                                         mybir.ActivationFunctionType.Copy)
                    nc.scalar.activation(em[:, 128:256], psB[:],
                                         mybir.ActivationFunctionType.Copy)
                    k = ce // cells_per_k
                    i0 = (ce % cells_per_k) * CELLR
                    nc.sync.dma_start(
                        part.ap()[k, :, i0:i0 + CELLR], em[:])

        # Layer 1
        scatter_layer(tab1.ap(), c.HID, c.HID, c.HID, part1)
        nc.gpsimd.collective_compute(
            "ReduceScatter", mybir.AluOpType.add, replica_groups=groups,
            ins=[part1.ap()], outs=[rs1.ap()])

        # Phase D: tab2 = relu(h1) @ W2 per block (rs1 already transposed)
        with tc.tile_pool(name="pd", bufs=3) as pd, \
             tc.tile_pool(name="psd", bufs=2, space="PSUM") as psd:
            for b in range(c.NB):
                h = pd.tile((c.HID, 128), f16, tag="h", name="h")
                nc.sync.dma_start(h[:], rs1.ap()[:, b * 128:(b + 1) * 128])
                hr = pd.tile((c.HID, 128), f16, tag="hr", name="hr")
                nc.vector.tensor_scalar_max(hr[:], h[:], 0.0)
                ps2 = psd.tile((128, c.OUT), f32, space="PSUM")
                nc.tensor.matmul(ps2[:], hr[:], w2_sb[:],
                                 start=True, stop=True)
                s2 = pd.tile((128, c.OUT), f16, tag="s2", name="s2")
                nc.scalar.activation(s2[:], ps2[:],
                                     mybir.ActivationFunctionType.Copy)
                nc.sync.dma_start(tab2.ap()[b * 128:(b + 1) * 128, 0:c.OUT],
                                  s2[:])

        # Layer 2 (64-wide rows, 128B gather elems at 256B stride)
        scatter_layer(tab2.ap()[:, 0:c.OUT], c.OUT, 128, c.OUT, part2)
        nc.gpsimd.collective_compute(
            "ReduceScatter", mybir.AluOpType.add, replica_groups=groups,
            ins=[part2.ap()], outs=[rs2.ap()])

        # Phase F: transpose rs2 -> out f32
        with tc.tile_pool(name="pf", bufs=3) as pf, \
             tc.tile_pool(name="psf", bufs=2, space="PSUM") as psf:
            for b in range(c.NB):
                t = pf.tile((c.OUT, 128), f16, tag="t", name="t")
                nc.sync.dma_start(t[:], rs2.ap()[:, b * 128:(b + 1) * 128])
                tp = psf.tile((128, c.OUT), f16, space="PSUM")
                nc.tensor.transpose(out=tp[:], in_=t[:],
                                    identity=ident_sb[0:c.OUT, 0:c.OUT])
                of = pf.tile((128, c.OUT), f32, tag="of", name="of")
                nc.vector.tensor_copy(of[:], tp[:])
                rows = min(128, c.NPC - b * 128)
                nc.sync.dma_start(out.ap()[b * 128:b * 128 + rows, :],
                                  of[0:rows, :])

    nc.compile()
    return nc


def make_inputs(cfg, features, edge_index, W1, W2):
    c = cfg
    colidx, rowloc2, meta = prep_edges(cfg, edge_index)
    iota2d = np.broadcast_to(np.arange(256, dtype=np.float16),
                             (128, 256)).copy()
    ident = np.eye(128, dtype=np.float16)
    w1 = np.ascontiguousarray(np.asarray(W1, np.float16))
    w2 = np.ascontiguousarray(np.asarray(W2, np.float16))
    in_maps = []
    for cc in range(c.NCORE):
        xc = np.asarray(features[cc * c.NPC:(cc + 1) * c.NPC], np.float32)
        xt = np.zeros((c.IN, c.NPP), np.float16)
        xt[:, :c.NPC] = xc.T.astype(np.float16)
        in_maps.append({
            "xT": np.ascontiguousarray(xt),
            "w1": w1, "w2": w2, "ident": ident, "iotaAB": iota2d,
            "colidx": np.ascontiguousarray(colidx[cc]),
            "rowloc2": np.ascontiguousarray(rowloc2[cc]),
        })
    return in_maps, meta


_LAST_NC = None


def kernel(features, edge_index, W1, W2):
    global _LAST_NC
    cfg = CFG
    in_maps, meta = make_inputs(cfg, features, edge_index, W1, W2)
    nc = build(cfg, meta)
    _LAST_NC = nc
    res = bass_utils.run_bass_kernel_spmd(
        nc, in_maps, core_ids=list(range(cfg.NCORE)))
    return np.concatenate(
        [res.results[cc]["out"] for cc in range(cfg.NCORE)], axis=0)


# revision 14
# speedup vs baseline: 1.4787x; 1.4787x over previous
"""2-layer GCN (gnn_message_passing) on 8 Trainium2 NeuronCores.

Strategy (v2, source-sharded + ReduceScatter):
  Edges are assigned to the core owning the SOURCE node (col). Each core:
    support1 = X_local @ W1 -> local table tab1 (12544 x 128 fp16, DRAM)
    scatter: for every global dest cell (256 rows = 2 blocks), gather
      sup rows by local col idx (dma_gather, int16 idx), one-hot matmul
      (transposed: psum = gt.T @ onehot -> (hid, dest)) into per-block psum,
      emit fp16 partial blocks into part1 (8, 128, 12544).
    ReduceScatter(add) -> rs1 (128, 12544): this core's h1 shard, transposed.
    h1 = relu(rs1); tab2 = h1.T @ W2 per block (no transpose needed).
    Same scatter for layer 2 (64-wide table rows, 128B gather elems),
    ReduceScatter -> rs2 (64, 12544), PE-transpose -> out f32.
  One-hot built with a parity trick (cell rows valued 0..255) so chunks can
  span the two blocks of a cell; rowloc stored duplicated x2 so the
  is_equal TensorTensor qualifies for the DVE 2x_1p fast mode.
"""
import sys
sys.path.insert(0, "/opt/trn_rl_repo")

import numpy as np
from contextlib import ExitStack

import concourse.bass as bass
import concourse.bacc as bacc
import concourse.tile as tile
from concourse import bass_utils
from concourse import mybir
from concourse.library_config import mlp

PADVAL = 300.0
GMAX = 32         # 128-idx chunks per dma_gather call
SCRATCH = 65536   # dynamic_dma_scratch_size (bytes; /16 = ring descs)
CELLR = 256       # dest rows per cell (2 blocks of 128)


class Config:
    def __init__(self, n=100000, in_dim=256, hid=128, out_dim=64, ncore=8):
        self.N = n
        self.IN = in_dim
        self.HID = hid
        self.OUT = out_dim
        self.NCORE = ncore
        self.NPC = n // ncore
        assert self.NPC * ncore == n
        self.NB = (self.NPC + 127) // 128
        self.NPP = self.NB * 128
        self.NT = ncore * self.NPP
        self.NCELL = self.NT // CELLR
        self.KT = in_dim // 128


CFG = Config()


def prep_edges(cfg, edge_index):
    """Bucket edges by (src core, dest cell); within a cell order by dest
    block (A = even block, B = odd). Chunk counts per cell are uniform
    across cores (max). Returns per-core colidx (int16, 16-wrapped),
    rowloc2 (fp16, duplicated x2), and compile-time cell metadata."""
    c = cfg
    row = np.asarray(edge_index[0], dtype=np.int64)
    col = np.asarray(edge_index[1], dtype=np.int64)
    src = col // c.NPC
    tloc = (col - src * c.NPC).astype(np.int16)
    rT = (row // c.NPC) * c.NPP + (row % c.NPC)
    cell = rT // CELLR
    par = (rT // 128) % 2
    rib = (rT % CELLR).astype(np.float16)  # 0..255, parity-coded row in cell

    key = (src * c.NCELL + cell) * 2 + par
    order = np.argsort(key, kind="stable")
    tloc_s = tloc[order]
    rib_s = rib[order]
    ncel2 = c.NCORE * c.NCELL * 2
    counts2 = np.bincount(key[order], minlength=ncel2).reshape(
        c.NCORE, c.NCELL, 2)
    cntA = counts2[:, :, 0]
    cntAB = counts2.sum(axis=2)
    starts = np.concatenate([[0], np.cumsum(counts2.reshape(-1))])

    cntB = counts2[:, :, 1]
    m_cell = np.maximum(-(-cntAB.max(axis=0) // 128), 1)       # chunks/cell
    chunksA = np.maximum(-(-cntA.max(axis=0) // 128), 1)
    bstart = np.maximum(m_cell - np.maximum(-(-cntB.max(axis=0) // 128), 1), 0)
    cbase = np.concatenate([[0], np.cumsum(m_cell)])
    nch = int(cbase[-1])

    colidx = np.zeros((c.NCORE, 128, nch * 8), np.int16)
    rowloc2 = np.full((c.NCORE, 128, nch * 2), PADVAL, np.float16)

    for cc in range(c.NCORE):
        for ce in range(c.NCELL):
            nk = int(m_cell[ce])
            cap = nk * 128
            kiA = (cc * c.NCELL + ce) * 2
            sA, sB, eB = starts[kiA], starts[kiA + 1], starts[kiA + 2]
            ca = int(sB - sA)
            cbn = int(eB - sB)
            tl = np.zeros(cap, np.int16)
            tl[:ca] = tloc_s[sA:sB]
            tl[cap - cbn:] = tloc_s[sB:eB]
            rb = np.full(cap, PADVAL, np.float16)
            rb[:ca] = rib_s[sA:sB]
            rb[cap - cbn:] = rib_s[sB:eB]
            qo = int(cbase[ce])
            colidx[cc][:, qo * 8:(qo + nk) * 8] = np.tile(
                tl.reshape(-1, 16).T, (8, 1))
            r2 = rb.reshape(nk, 128).T  # (128, nk)
            rowloc2[cc][:, qo * 2:(qo + nk) * 2] = np.repeat(r2, 2, axis=1)
    meta = dict(m_cell=m_cell, chunksA=chunksA, bstart=bstart,
                cbase=cbase, nch=nch)
    return colidx, rowloc2, meta


def _dma_gather_raw(gp, out_ap, in_ap, idxs_ap, num_idxs, num_idxs_reg,
                    elem_size, elem_step, queue_num):
    """dma_gather (non-transpose, DRAM source) without the %256 payload
    assert — the ucode only requires 256B multiples for transpose mode;
    the source ROW STRIDE must still be a 256B multiple."""
    gp._assert_queue_num(queue_num)
    assert idxs_ap.dtype == mybir.dt.int16
    assert in_ap.dtype == out_ap.dtype
    elem_size_bytes = elem_size * mybir.dt.size(in_ap.dtype)
    assert elem_size_bytes > 0 and elem_size_bytes % 128 == 0
    assert in_ap.space == bass.MemorySpace.DRAM
    assert idxs_ap.space == bass.MemorySpace.SBUF
    assert out_ap.space == bass.MemorySpace.SBUF
    assert in_ap.ap[-1][1] == out_ap.ap[-1][1] == elem_size
    assert out_ap.ap[0][1] * out_ap.ap[1][1] == ((num_idxs + 127) // 128) * 128
    assert in_ap.ap[0][0] == elem_step
    stride_bytes = elem_step * mybir.dt.size(in_ap.dtype)
    stride_bytes_256 = stride_bytes // 256
    assert stride_bytes_256 * 256 == stride_bytes and stride_bytes_256 < 256
    _in_ap = gp.lower_ap_dma(in_ap, for_custom_bir_dma=True)
    _idxs_ap = gp.lower_ap(idxs_ap)
    _out_ap = gp.lower_ap(out_ap)
    return gp.add_instruction(
        mybir.InstDMAGatherAnt(
            name=gp.bass.get_next_instruction_name(),
            ins=[*_in_ap, _idxs_ap,
                 gp.lower_val_access(gp.to_reg(num_idxs_reg))],
            outs=[_out_ap],
            transpose=False, num_idxs=num_idxs, elem_size=elem_size,
            stride_bytes_256=stride_bytes_256, gen_mode=0,
            single_packet=True, queue_num=queue_num,
            sbuf_tokens_per_rank=0, sbuf_free_dim_per_rank=0,
            sbuf_free_dim_pad_per_rank=0, sbuf_byte_offset=0))


def build(cfg, meta):
    c = cfg
    m_cell, chunksA, bstart, cbase, nch = (
        meta["m_cell"], meta["chunksA"], meta["bstart"], meta["cbase"],
        meta["nch"])
    nc = bacc.Bacc(None, target_bir_lowering=False, debug=False,
                   num_devices=c.NCORE, name="gcn2", num_swdge_queues=4,
                   dynamic_dma_scratch_size=SCRATCH)
    f16, f32, i16 = mybir.dt.float16, mybir.dt.float32, mybir.dt.int16
    iseq = mybir.AluOpType.is_equal

    xT = nc.dram_tensor("xT", (c.IN, c.NPP), f16, kind="ExternalInput")
    w1 = nc.dram_tensor("w1", (c.IN, c.HID), f16, kind="ExternalInput")
    w2 = nc.dram_tensor("w2", (c.HID, c.OUT), f16, kind="ExternalInput")
    ident = nc.dram_tensor("ident", (128, 128), f16, kind="ExternalInput")
    iotaAB = nc.dram_tensor("iotaAB", (128, 256), f16, kind="ExternalInput")
    colidx = nc.dram_tensor("colidx", (128, nch * 8), i16,
                            kind="ExternalInput")
    rowloc2 = nc.dram_tensor("rowloc2", (128, nch * 2), f16,
                             kind="ExternalInput")
    out = nc.dram_tensor("out", (c.NPC, c.OUT), f32, kind="ExternalOutput")

    tab1 = nc.dram_tensor("tab1", (c.NPP, c.HID), f16, kind="Internal")
    part1 = nc.dram_tensor("part1", (c.NCORE, c.HID, c.NPP), f16,
                           kind="Internal")
    rs1 = nc.dram_tensor("rs1", (c.HID, c.NPP), f16, kind="Internal")
    tab2 = nc.dram_tensor("tab2", (c.NPP, 128), f16, kind="Internal")
    part2 = nc.dram_tensor("part2", (c.NCORE, c.OUT, c.NPP), f16,
                           kind="Internal")
    rs2 = nc.dram_tensor("rs2", (c.OUT, c.NPP), f16, kind="Internal")

    groups = [list(range(c.NCORE))]
    cells_per_k = c.NPP // CELLR  # dest cells per core shard

    with ExitStack() as ctx:
        tc = ctx.enter_context(tile.TileContext(nc))
        nc.gpsimd.load_library(mlp)
        cpool = ctx.enter_context(tc.tile_pool(name="const", bufs=1))
        ident_sb = cpool.tile((128, 128), f16, tag="ident")
        nc.sync.dma_start(ident_sb[:], ident[:])
        iota_sb = cpool.tile((128, 256), f16, tag="iota")
        nc.sync.dma_start(iota_sb[:], iotaAB[:])
        w2_sb = cpool.tile((c.HID, c.OUT), f16, tag="w2")
        nc.sync.dma_start(w2_sb[:], w2[:])
        rl2_sb = cpool.tile((128, nch * 2), f16, tag="rl2")
        nc.sync.dma_start(rl2_sb[:], rowloc2[:])

        # Phase A: tab1 = X @ W1 per block
        with tc.tile_pool(name="pa", bufs=1) as pa, \
             tc.tile_pool(name="pas", bufs=3) as pas, \
             tc.tile_pool(name="psa", bufs=2, space="PSUM") as psa:
            xk = []
            w1k = []
            for k in range(c.KT):
                t = pa.tile((128, c.NPP), f16, tag=f"x{k}", name=f"xk{k}")
                nc.sync.dma_start(t[:], xT.ap()[k * 128:(k + 1) * 128, :])
                xk.append(t)
                t = pa.tile((128, c.HID), f16, tag=f"w{k}", name=f"w1k{k}")
                nc.sync.dma_start(t[:], w1.ap()[k * 128:(k + 1) * 128, :])
                w1k.append(t)
            s1g = None
            for b in range(c.NB):
                ps = psa.tile((128, c.HID), f32, space="PSUM")
                for k in range(c.KT):
                    nc.tensor.matmul(ps[:], xk[k][:, b * 128:(b + 1) * 128],
                                     w1k[k][:], start=(k == 0),
                                     stop=(k == c.KT - 1))
                q = b % 4
                if q == 0:
                    s1g = pas.tile((128, 4, c.HID), f16)
                nc.scalar.activation(s1g[:, q, :], ps[:],
                                     mybir.ActivationFunctionType.Copy)
                if q == 3 or b == c.NB - 1:
                    b0 = b - q
                    x = s1g[:, 0:q + 1, :]
                    dst = bass.AP(tab1, b0 * 128 * c.HID,
                                  [[c.HID, 128], [128 * c.HID, q + 1],
                                   [1, c.HID]])
                    nc.sync.dma_start(dst, x)

        def scatter_layer(tab_ap, elem, estep, W, part):
            """Gather+scatter all dest cells. Table rows `elem` fp16 payload
            at stride `estep`; W = feature width; psum blocks transposed
            (W, 128); partial cells (W, 256) -> part (NCORE, W, NPP)."""
            gq = [0]
            call_tile = {}
            seg_tile = {}
            em_cur = [None]
            ncall = -(-nch // GMAX)
            CPS = 64  # gather calls per colidx segment DMA
            SEGCH = CPS * GMAX

            with tc.tile_pool(name="poh", bufs=4) as poh, \
                 tc.tile_pool(name="pg", bufs=6) as pg, \
                 tc.tile_pool(name="pix", bufs=2) as pix, \
                 tc.tile_pool(name="pem", bufs=3) as pem, \
                 tc.tile_pool(name="pso", bufs=3, space="PSUM") as pso:

                def ensure_call(q):
                    if q in call_tile:
                        return call_tile[q]
                    s = q // CPS
                    if s not in seg_tile:
                        ch0 = s * SEGCH
                        chn = min(SEGCH, nch - ch0)
                        st = pix.tile((128, chn * 8), i16)
                        nc.sync.dma_start(
                            st[:], colidx.ap()[:, ch0 * 8:(ch0 + chn) * 8])
                        seg_tile[s] = st
                    st = seg_tile[s]
                    sn = min(GMAX, nch - q * GMAX)
                    off = (q * GMAX - s * SEGCH) * 8
                    gt = pg.tile((128, sn, elem), f16)
                    _dma_gather_raw(
                        nc.gpsimd, gt[:], tab_ap, st[:, off:off + sn * 8],
                        sn * 128, sn * 128, elem, estep, gq[0] % 4)
                    gq[0] += 1
                    call_tile[q] = (gt, sn)
                    return call_tile[q]

                for ce in range(c.NCELL):
                    cb = int(cbase[ce])
                    m = int(m_cell[ce])
                    cA = int(chunksA[ce])
                    bs = int(bstart[ce])
                    ncols = cA + (m - bs)
                    oh = poh.tile((128, ncols, 128), f16)
                    # one-hot: A-cols vs iota 0..127, B-cols vs 128..255
                    for (c0, c1, pslice, j0) in (
                            (0, cA, 0, 0), (cA, ncols, 1, bs)):
                        nco = c1 - c0
                        ox = oh[:, c0:c1, :]
                        ov = bass.AP(ox.tensor, ox.offset,
                                     [ox.ap[0], [128, nco], [2, 64], [1, 2]])
                        ix = iota_sb[:, pslice * 128:(pslice + 1) * 128]
                        iv = bass.AP(ix.tensor, ix.offset,
                                     [ix.ap[0], [0, nco], [2, 64], [1, 2]])
                        rx = rl2_sb[:, 2 * (cb + j0):2 * (cb + j0) + 2 * nco]
                        rv = bass.AP(rx.tensor, rx.offset,
                                     [rx.ap[0], [2, nco], [0, 64], [1, 2]])
                        nc.vector.tensor_tensor(out=ov, in0=iv, in1=rv,
                                                op=iseq)
                    psA = pso.tile((W, 128), f32, space="PSUM", tag="psA",
                                   name="psA")
                    psB = pso.tile((W, 128), f32, space="PSUM", tag="psB",
                                   name="psB")
                    for j in range(m):
                        gi = cb + j
                        gt, sn = ensure_call(gi // GMAX)
                        sl = gt[:, gi % GMAX, 0:W]
                        if j < cA:
                            nc.tensor.matmul(psA[:], sl, oh[:, j, :],
                                             start=(j == 0),
                                             stop=(j == cA - 1))
                        if j >= bs:
                            nc.tensor.matmul(psB[:], sl,
                                             oh[:, cA + (j - bs), :],
                                             start=(j == bs),
                                             stop=(j == m - 1))
                    k = ce // cells_per_k
                    loc = ce % cells_per_k
                    if loc % 2 == 0:
                        em_cur[0] = pem.tile((W, 2 * CELLR), f16, name="em", tag="em")
                    em = em_cur[0]
                    e0 = (loc % 2) * CELLR
                    nc.scalar.activation(em[:, e0:e0 + 128], psA[:],
                                         mybir.ActivationFunctionType.Copy)
                    nc.scalar.activation(em[:, e0 + 128:e0 + 256], psB[:],
                                         mybir.ActivationFunctionType.Copy)
                    if loc % 2 == 1 or loc == cells_per_k - 1:
                        i0 = (loc - loc % 2) * CELLR
                        nc.sync.dma_start(
                            part.ap()[k, :, i0:i0 + e0 + CELLR],
                            em[:, 0:e0 + CELLR])

        # Layer 1
        scatter_layer(tab1.ap(), c.HID, c.HID, c.HID, part1)
        nc.gpsimd.collective_compute(
            "ReduceScatter", mybir.AluOpType.add, replica_groups=groups,
            ins=[part1.ap()], outs=[rs1.ap()])

        # Phase D: tab2 = relu(h1) @ W2, 4 blocks per DMA (rs1 pre-transposed)
        with tc.tile_pool(name="pd", bufs=3) as pd, \
             tc.tile_pool(name="psd", bufs=4, space="PSUM") as psd:
            for b0 in range(0, c.NB, 4):
                nb4 = min(4, c.NB - b0)
                h = pd.tile((c.HID, nb4 * 128), f16, tag="h", name="h")
                nc.sync.dma_start(
                    h[:], rs1.ap()[:, b0 * 128:(b0 + nb4) * 128])
                hr = pd.tile((c.HID, nb4 * 128), f16, tag="hr", name="hr")
                nc.vector.tensor_scalar_max(hr[:], h[:], 0.0)
                s2 = pd.tile((128, nb4, c.OUT), f16, tag="s2", name="s2")
                for q in range(nb4):
                    ps2 = psd.tile((128, c.OUT), f32, space="PSUM")
                    nc.tensor.matmul(ps2[:], hr[:, q * 128:(q + 1) * 128],
                                     w2_sb[:], start=True, stop=True)
                    nc.scalar.activation(s2[:, q, :], ps2[:],
                                         mybir.ActivationFunctionType.Copy)
                dst = bass.AP(tab2, b0 * 128 * 128,
                              [[128, 128], [128 * 128, nb4], [1, c.OUT]])
                nc.sync.dma_start(dst, s2[:])

        # Layer 2 (64-wide rows, 128B gather elems at 256B stride)
        scatter_layer(tab2.ap()[:, 0:c.OUT], c.OUT, 128, c.OUT, part2)
        nc.gpsimd.collective_compute(
            "ReduceScatter", mybir.AluOpType.add, replica_groups=groups,
            ins=[part2.ap()], outs=[rs2.ap()])

        # Phase F: transpose rs2 -> out f32, 4 blocks per DMA
        with tc.tile_pool(name="pf", bufs=3) as pf, \
             tc.tile_pool(name="psf", bufs=4, space="PSUM") as psf:
            for b0 in range(0, c.NB, 4):
                nb4 = min(4, c.NB - b0)
                t = pf.tile((c.OUT, nb4 * 128), f16, tag="t", name="t")
                nc.sync.dma_start(
                    t[:], rs2.ap()[:, b0 * 128:(b0 + nb4) * 128])
                of = pf.tile((128, nb4, c.OUT), f32, tag="of", name="of")
                for q in range(nb4):
                    tp = psf.tile((128, c.OUT), f16, space="PSUM")
                    nc.tensor.transpose(out=tp[:],
                                        in_=t[:, q * 128:(q + 1) * 128],
                                        identity=ident_sb[0:c.OUT, 0:c.OUT])
                    nc.vector.tensor_copy(of[:, q, :], tp[:])
                rows = min(4 * 128, c.NPC - b0 * 128)
                nfull = rows // 128
                if nfull:
                    dst = bass.AP(out, b0 * 128 * c.OUT,
                                  [[c.OUT, 128], [128 * c.OUT, nfull],
                                   [1, c.OUT]])
                    nc.sync.dma_start(dst, of[:, 0:nfull, :])
                rem = rows - nfull * 128
                if rem:
                    nc.sync.dma_start(
                        out.ap()[(b0 + nfull) * 128:
                                 (b0 + nfull) * 128 + rem, :],
                        of[0:rem, nfull, :])

    nc.compile()
    return nc


def make_inputs(cfg, features, edge_index, W1, W2):
    c = cfg
    colidx, rowloc2, meta = prep_edges(cfg, edge_index)
    iota2d = np.broadcast_to(np.arange(256, dtype=np.float16),
                             (128, 256)).copy()
    ident = np.eye(128, dtype=np.float16)
    w1 = np.ascontiguousarray(np.asarray(W1, np.float16))
    w2 = np.ascontiguousarray(np.asarray(W2, np.float16))
    in_maps = []
    for cc in range(c.NCORE):
        xc = np.asarray(features[cc * c.NPC:(cc + 1) * c.NPC], np.float32)
        xt = np.zeros((c.IN, c.NPP), np.float16)
        xt[:, :c.NPC] = xc.T.astype(np.float16)
        in_maps.append({
            "xT": np.ascontiguousarray(xt),
            "w1": w1, "w2": w2, "ident": ident, "iotaAB": iota2d,
            "colidx": np.ascontiguousarray(colidx[cc]),
            "rowloc2": np.ascontiguousarray(rowloc2[cc]),
        })
    return in_maps, meta


_LAST_NC = None


def kernel(features, edge_index, W1, W2):
    global _LAST_NC
    cfg = CFG
    in_maps, meta = make_inputs(cfg, features, edge_index, W1, W2)
    nc = build(cfg, meta)
    _LAST_NC = nc
    res = bass_utils.run_bass_kernel_spmd(
        nc, in_maps, core_ids=list(range(cfg.NCORE)))
    return np.concatenate(
        [res.results[cc]["out"] for cc in range(cfg.NCORE)], axis=0)


# revision 15
# speedup vs baseline: 1.8758x; 1.2685x over previous
"""2-layer GCN (gnn_message_passing) on 8 Trainium2 NeuronCores.

Strategy (v2, source-sharded + ReduceScatter):
  Edges are assigned to the core owning the SOURCE node (col). Each core:
    support1 = X_local @ W1 -> local table tab1 (12544 x 128 fp16, DRAM)
    scatter: for every global dest cell (256 rows = 2 blocks), gather
      sup rows by local col idx (dma_gather, int16 idx), one-hot matmul
      (transposed: psum = gt.T @ onehot -> (hid, dest)) into per-block psum,
      emit fp16 partial blocks into part1 (8, 128, 12544).
    ReduceScatter(add) -> rs1 (128, 12544): this core's h1 shard, transposed.
    h1 = relu(rs1); tab2 = h1.T @ W2 per block (no transpose needed).
    Same scatter for layer 2 (64-wide table rows, 128B gather elems),
    ReduceScatter -> rs2 (64, 12544), PE-transpose -> out f32.
  One-hot built with a parity trick (cell rows valued 0..255) so chunks can
  span the two blocks of a cell; rowloc stored duplicated x2 so the
  is_equal TensorTensor qualifies for the DVE 2x_1p fast mode.
"""
import sys
sys.path.insert(0, "/opt/trn_rl_repo")

import numpy as np
from contextlib import ExitStack

import concourse.bass as bass
import concourse.bacc as bacc
import concourse.tile as tile
from concourse import bass_utils
from concourse import mybir
from concourse.library_config import mlp

PADVAL = 300.0
GMAX = 32         # 128-idx chunks per dma_gather call
SCRATCH = 65536   # dynamic_dma_scratch_size (bytes; /16 = ring descs)
CELLR = 256       # dest rows per cell (2 blocks of 128)


class Config:
    def __init__(self, n=100000, in_dim=256, hid=128, out_dim=64, ncore=8):
        self.N = n
        self.IN = in_dim
        self.HID = hid
        self.OUT = out_dim
        self.NCORE = ncore
        self.NPC = n // ncore
        assert self.NPC * ncore == n
        self.NB = (self.NPC + 127) // 128
        self.NPP = self.NB * 128
        self.NT = ncore * self.NPP
        self.NCELL = self.NT // CELLR
        self.KT = in_dim // 128


CFG = Config()


def prep_edges(cfg, edge_index):
    """Bucket edges by (src core, dest cell); within a cell order by dest
    block (A = even block, B = odd). Chunk counts per cell are uniform
    across cores (max). Returns per-core colidx (int16, 16-wrapped),
    rowloc2 (fp16, duplicated x2), and compile-time cell metadata."""
    c = cfg
    row = np.asarray(edge_index[0], dtype=np.int64)
    col = np.asarray(edge_index[1], dtype=np.int64)
    src = col // c.NPC
    tloc = (col - src * c.NPC).astype(np.int16)
    rT = (row // c.NPC) * c.NPP + (row % c.NPC)
    cell = rT // CELLR
    par = (rT // 128) % 2
    rib = (rT % CELLR).astype(np.float16)  # 0..255, parity-coded row in cell

    key = (src * c.NCELL + cell) * 2 + par
    order = np.argsort(key, kind="stable")
    tloc_s = tloc[order]
    rib_s = rib[order]
    ncel2 = c.NCORE * c.NCELL * 2
    counts2 = np.bincount(key[order], minlength=ncel2).reshape(
        c.NCORE, c.NCELL, 2)
    cntA = counts2[:, :, 0]
    cntAB = counts2.sum(axis=2)
    starts = np.concatenate([[0], np.cumsum(counts2.reshape(-1))])

    cntB = counts2[:, :, 1]
    m_cell = np.maximum(-(-cntAB.max(axis=0) // 128), 1)       # chunks/cell
    chunksA = np.maximum(-(-cntA.max(axis=0) // 128), 1)
    bstart = np.maximum(m_cell - np.maximum(-(-cntB.max(axis=0) // 128), 1), 0)
    cbase = np.concatenate([[0], np.cumsum(m_cell)])
    nch = int(cbase[-1])

    colidx = np.zeros((c.NCORE, 128, nch * 8), np.int16)
    rowloc2 = np.full((c.NCORE, 128, nch * 2), PADVAL, np.float16)

    for cc in range(c.NCORE):
        for ce in range(c.NCELL):
            nk = int(m_cell[ce])
            cap = nk * 128
            kiA = (cc * c.NCELL + ce) * 2
            sA, sB, eB = starts[kiA], starts[kiA + 1], starts[kiA + 2]
            ca = int(sB - sA)
            cbn = int(eB - sB)
            tl = np.zeros(cap, np.int16)
            tl[:ca] = tloc_s[sA:sB]
            tl[cap - cbn:] = tloc_s[sB:eB]
            rb = np.full(cap, PADVAL, np.float16)
            rb[:ca] = rib_s[sA:sB]
            rb[cap - cbn:] = rib_s[sB:eB]
            qo = int(cbase[ce])
            colidx[cc][:, qo * 8:(qo + nk) * 8] = np.tile(
                tl.reshape(-1, 16).T, (8, 1))
            r2 = rb.reshape(nk, 128).T  # (128, nk)
            rowloc2[cc][:, qo * 2:(qo + nk) * 2] = np.repeat(r2, 2, axis=1)
    meta = dict(m_cell=m_cell, chunksA=chunksA, bstart=bstart,
                cbase=cbase, nch=nch)
    return colidx, rowloc2, meta


def _dma_gather_raw(gp, out_ap, in_ap, idxs_ap, num_idxs, num_idxs_reg,
                    elem_size, elem_step, queue_num):
    """dma_gather (non-transpose, DRAM source) without the %256 payload
    assert — the ucode only requires 256B multiples for transpose mode;
    the source ROW STRIDE must still be a 256B multiple."""
    gp._assert_queue_num(queue_num)
    assert idxs_ap.dtype == mybir.dt.int16
    assert in_ap.dtype == out_ap.dtype
    elem_size_bytes = elem_size * mybir.dt.size(in_ap.dtype)
    assert elem_size_bytes > 0 and elem_size_bytes % 128 == 0
    assert in_ap.space == bass.MemorySpace.DRAM
    assert idxs_ap.space == bass.MemorySpace.SBUF
    assert out_ap.space == bass.MemorySpace.SBUF
    assert in_ap.ap[-1][1] == out_ap.ap[-1][1] == elem_size
    assert out_ap.ap[0][1] * out_ap.ap[1][1] == ((num_idxs + 127) // 128) * 128
    assert in_ap.ap[0][0] == elem_step
    stride_bytes = elem_step * mybir.dt.size(in_ap.dtype)
    stride_bytes_256 = stride_bytes // 256
    assert stride_bytes_256 * 256 == stride_bytes and stride_bytes_256 < 256
    _in_ap = gp.lower_ap_dma(in_ap, for_custom_bir_dma=True)
    _idxs_ap = gp.lower_ap(idxs_ap)
    _out_ap = gp.lower_ap(out_ap)
    return gp.add_instruction(
        mybir.InstDMAGatherAnt(
            name=gp.bass.get_next_instruction_name(),
            ins=[*_in_ap, _idxs_ap,
                 gp.lower_val_access(gp.to_reg(num_idxs_reg))],
            outs=[_out_ap],
            transpose=False, num_idxs=num_idxs, elem_size=elem_size,
            stride_bytes_256=stride_bytes_256, gen_mode=0,
            single_packet=True, queue_num=queue_num,
            sbuf_tokens_per_rank=0, sbuf_free_dim_per_rank=0,
            sbuf_free_dim_pad_per_rank=0, sbuf_byte_offset=0))


def build(cfg, meta):
    c = cfg
    m_cell, chunksA, bstart, cbase, nch = (
        meta["m_cell"], meta["chunksA"], meta["bstart"], meta["cbase"],
        meta["nch"])
    nc = bacc.Bacc(None, target_bir_lowering=False, debug=False,
                   num_devices=c.NCORE, name="gcn2", num_swdge_queues=4,
                   dynamic_dma_scratch_size=SCRATCH)
    f16, f32, i16 = mybir.dt.float16, mybir.dt.float32, mybir.dt.int16
    iseq = mybir.AluOpType.is_equal

    xT = nc.dram_tensor("xT", (c.IN, c.NPP), f16, kind="ExternalInput")
    w1 = nc.dram_tensor("w1", (c.IN, c.HID), f16, kind="ExternalInput")
    w2 = nc.dram_tensor("w2", (c.HID, c.OUT), f16, kind="ExternalInput")
    ident = nc.dram_tensor("ident", (128, 128), f16, kind="ExternalInput")
    iotaAB = nc.dram_tensor("iotaAB", (128, 256), f16, kind="ExternalInput")
    colidx = nc.dram_tensor("colidx", (128, nch * 8), i16,
                            kind="ExternalInput")
    rowloc2 = nc.dram_tensor("rowloc2", (128, nch * 2), f16,
                             kind="ExternalInput")
    out = nc.dram_tensor("out", (c.NPC, c.OUT), f32, kind="ExternalOutput")

    f8 = mybir.dt.float8e4
    tab1 = nc.dram_tensor("tab1", (c.NPP, 2 * c.HID), f8, kind="Internal")
    part1 = nc.dram_tensor("part1", (c.NCORE, c.HID, c.NPP), f16,
                           kind="Internal")
    rs1 = nc.dram_tensor("rs1", (c.HID, c.NPP), f16, kind="Internal")
    tab2 = nc.dram_tensor("tab2", (c.NPP, 128), f16, kind="Internal")
    part2 = nc.dram_tensor("part2", (c.NCORE, c.OUT, c.NPP), f16,
                           kind="Internal")
    rs2 = nc.dram_tensor("rs2", (c.OUT, c.NPP), f16, kind="Internal")

    groups = [list(range(c.NCORE))]
    cells_per_k = c.NPP // CELLR  # dest cells per core shard

    with ExitStack() as ctx:
        tc = ctx.enter_context(tile.TileContext(nc))
        nc.gpsimd.load_library(mlp)
        cpool = ctx.enter_context(tc.tile_pool(name="const", bufs=1))
        ident_sb = cpool.tile((128, 128), f16, tag="ident")
        nc.sync.dma_start(ident_sb[:], ident[:])
        iota_sb = cpool.tile((128, 256), f16, tag="iota")
        nc.sync.dma_start(iota_sb[:], iotaAB[:])
        w2_sb = cpool.tile((c.HID, c.OUT), f16, tag="w2")
        nc.sync.dma_start(w2_sb[:], w2[:])
        rl2_sb = cpool.tile((128, nch * 2), f16, tag="rl2")
        nc.sync.dma_start(rl2_sb[:], rowloc2[:])

        # Phase A: tab1 = X @ W1 per block
        with tc.tile_pool(name="pa", bufs=1) as pa, \
             tc.tile_pool(name="pas", bufs=3) as pas, \
             tc.tile_pool(name="psa", bufs=2, space="PSUM") as psa:
            xk = []
            w1k = []
            for k in range(c.KT):
                t = pa.tile((128, c.NPP), f16, tag=f"x{k}", name=f"xk{k}")
                nc.sync.dma_start(t[:], xT.ap()[k * 128:(k + 1) * 128, :])
                xk.append(t)
                t = pa.tile((128, c.HID), f16, tag=f"w{k}", name=f"w1k{k}")
                nc.sync.dma_start(t[:], w1.ap()[k * 128:(k + 1) * 128, :])
                w1k.append(t)
            s1g = None
            for b in range(c.NB):
                ps = psa.tile((128, c.HID), f32, space="PSUM")
                for k in range(c.KT):
                    nc.tensor.matmul(ps[:], xk[k][:, b * 128:(b + 1) * 128],
                                     w1k[k][:], start=(k == 0),
                                     stop=(k == c.KT - 1))
                q = b % 4
                if q == 0:
                    s1g = pas.tile((128, 4, c.HID), f8, name="s1g", tag="s1g")
                nc.scalar.activation(s1g[:, q, :], ps[:],
                                     mybir.ActivationFunctionType.Copy)
                if q == 3 or b == c.NB - 1:
                    b0 = b - q
                    x = s1g[:, 0:q + 1, :]
                    dst = bass.AP(tab1, b0 * 128 * 2 * c.HID,
                                  [[2 * c.HID, 128], [128 * 2 * c.HID, q + 1],
                                   [1, c.HID]])
                    nc.sync.dma_start(dst, x)

        def scatter_layer(tab_ap, elem, estep, W, part, gdt):
            """Gather+scatter all dest cells. Table rows `elem` fp16 payload
            at stride `estep`; W = feature width; psum blocks transposed
            (W, 128); partial cells (W, 256) -> part (NCORE, W, NPP)."""
            gq = [0]
            call_tile = {}
            seg_tile = {}
            ps_cur = [None]
            ncall = -(-nch // GMAX)
            CPS = 64  # gather calls per colidx segment DMA
            SEGCH = CPS * GMAX

            with tc.tile_pool(name="poh", bufs=4) as poh, \
                 tc.tile_pool(name="pg", bufs=6) as pg, \
                 tc.tile_pool(name="pix", bufs=2) as pix, \
                 tc.tile_pool(name="pem", bufs=3) as pem, \
                 tc.tile_pool(name="pso", bufs=3, space="PSUM") as pso:

                def ensure_call(q):
                    if q in call_tile:
                        return call_tile[q]
                    s = q // CPS
                    if s not in seg_tile:
                        ch0 = s * SEGCH
                        chn = min(SEGCH, nch - ch0)
                        st = pix.tile((128, chn * 8), i16)
                        nc.sync.dma_start(
                            st[:], colidx.ap()[:, ch0 * 8:(ch0 + chn) * 8])
                        seg_tile[s] = st
                    st = seg_tile[s]
                    sn = min(GMAX, nch - q * GMAX)
                    off = (q * GMAX - s * SEGCH) * 8
                    gt = pg.tile((128, sn, elem), gdt, name="gt", tag="gt")
                    _dma_gather_raw(
                        nc.gpsimd, gt[:], tab_ap, st[:, off:off + sn * 8],
                        sn * 128, sn * 128, elem, estep, gq[0] % 4)
                    gq[0] += 1
                    call_tile[q] = (gt, sn)
                    return call_tile[q]

                for ce in range(c.NCELL):
                    cb = int(cbase[ce])
                    m = int(m_cell[ce])
                    cA = int(chunksA[ce])
                    bs = int(bstart[ce])
                    ncols = cA + (m - bs)
                    oh = poh.tile((128, ncols, 128), f16)
                    # one-hot: A-cols vs iota 0..127, B-cols vs 128..255
                    for (c0, c1, pslice, j0) in (
                            (0, cA, 0, 0), (cA, ncols, 1, bs)):
                        nco = c1 - c0
                        ox = oh[:, c0:c1, :]
                        ov = bass.AP(ox.tensor, ox.offset,
                                     [ox.ap[0], [128, nco], [2, 64], [1, 2]])
                        ix = iota_sb[:, pslice * 128:(pslice + 1) * 128]
                        iv = bass.AP(ix.tensor, ix.offset,
                                     [ix.ap[0], [0, nco], [2, 64], [1, 2]])
                        rx = rl2_sb[:, 2 * (cb + j0):2 * (cb + j0) + 2 * nco]
                        rv = bass.AP(rx.tensor, rx.offset,
                                     [rx.ap[0], [2, nco], [0, 64], [1, 2]])
                        nc.vector.tensor_tensor(out=ov, in0=iv, in1=rv,
                                                op=iseq)
                    k = ce // cells_per_k
                    loc = ce % cells_per_k
                    if loc % 2 == 0:
                        ps_cur[0] = pso.tile((W, 512), f32, space="PSUM",
                                             name="psAB", tag="psAB")
                    p0 = (loc % 2) * 256
                    psA = ps_cur[0][:, p0:p0 + 128]
                    psB = ps_cur[0][:, p0 + 128:p0 + 256]
                    for j in range(m):
                        gi = cb + j
                        gt, sn = ensure_call(gi // GMAX)
                        sl = gt[:, gi % GMAX, 0:W]
                        if j < cA:
                            nc.tensor.matmul(psA, sl, oh[:, j, :],
                                             start=(j == 0),
                                             stop=(j == cA - 1))
                        if j >= bs:
                            nc.tensor.matmul(psB, sl,
                                             oh[:, cA + (j - bs), :],
                                             start=(j == bs),
                                             stop=(j == m - 1))
                    if loc % 2 == 1 or loc == cells_per_k - 1:
                        w = p0 + 256
                        em = pem.tile((W, w), f16, name="em", tag="em")
                        nc.scalar.activation(
                            em[:], ps_cur[0][:, 0:w],
                            mybir.ActivationFunctionType.Copy)
                        i0 = (loc - loc % 2) * CELLR
                        nc.sync.dma_start(
                            part.ap()[k, :, i0:i0 + w], em[:])

        # Layer 1
        scatter_layer(tab1.ap()[:, 0:c.HID], c.HID, 2 * c.HID, c.HID,
                      part1, f8)
        nc.gpsimd.collective_compute(
            "ReduceScatter", mybir.AluOpType.add, replica_groups=groups,
            ins=[part1.ap()], outs=[rs1.ap()])

        # Phase D: tab2 = relu(h1) @ W2, 4 blocks per DMA (rs1 pre-transposed)
        with tc.tile_pool(name="pd", bufs=3) as pd, \
             tc.tile_pool(name="psd", bufs=4, space="PSUM") as psd:
            for b0 in range(0, c.NB, 4):
                nb4 = min(4, c.NB - b0)
                h = pd.tile((c.HID, nb4 * 128), f16, tag="h", name="h")
                nc.sync.dma_start(
                    h[:], rs1.ap()[:, b0 * 128:(b0 + nb4) * 128])
                hr = pd.tile((c.HID, nb4 * 128), f16, tag="hr", name="hr")
                nc.vector.tensor_scalar_max(hr[:], h[:], 0.0)
                s2 = pd.tile((128, nb4, c.OUT), f16, tag="s2", name="s2")
                for q in range(nb4):
                    ps2 = psd.tile((128, c.OUT), f32, space="PSUM")
                    nc.tensor.matmul(ps2[:], hr[:, q * 128:(q + 1) * 128],
                                     w2_sb[:], start=True, stop=True)
                    nc.scalar.activation(s2[:, q, :], ps2[:],
                                         mybir.ActivationFunctionType.Copy)
                dst = bass.AP(tab2, b0 * 128 * 128,
                              [[128, 128], [128 * 128, nb4], [1, c.OUT]])
                nc.sync.dma_start(dst, s2[:])

        # Layer 2 (64-wide rows, 128B gather elems at 256B stride)
        scatter_layer(tab2.ap()[:, 0:c.OUT], c.OUT, 128, c.OUT, part2, f16)
        nc.gpsimd.collective_compute(
            "ReduceScatter", mybir.AluOpType.add, replica_groups=groups,
            ins=[part2.ap()], outs=[rs2.ap()])

        # Phase F: transpose rs2 -> out f32, 4 blocks per DMA
        with tc.tile_pool(name="pf", bufs=3) as pf, \
             tc.tile_pool(name="psf", bufs=4, space="PSUM") as psf:
            for b0 in range(0, c.NB, 4):
                nb4 = min(4, c.NB - b0)
                t = pf.tile((c.OUT, nb4 * 128), f16, tag="t", name="t")
                nc.sync.dma_start(
                    t[:], rs2.ap()[:, b0 * 128:(b0 + nb4) * 128])
                of = pf.tile((128, nb4, c.OUT), f32, tag="of", name="of")
                for q in range(nb4):
                    tp = psf.tile((128, c.OUT), f16, space="PSUM")
                    nc.tensor.transpose(out=tp[:],
                                        in_=t[:, q * 128:(q + 1) * 128],
                                        identity=ident_sb[0:c.OUT, 0:c.OUT])
                    nc.vector.tensor_copy(of[:, q, :], tp[:])
                rows = min(4 * 128, c.NPC - b0 * 128)
                nfull = rows // 128
                if nfull:
                    dst = bass.AP(out, b0 * 128 * c.OUT,
                                  [[c.OUT, 128], [128 * c.OUT, nfull],
                                   [1, c.OUT]])
                    nc.sync.dma_start(dst, of[:, 0:nfull, :])
                rem = rows - nfull * 128
                if rem:
                    nc.sync.dma_start(
                        out.ap()[(b0 + nfull) * 128:
                                 (b0 + nfull) * 128 + rem, :],
                        of[0:rem, nfull, :])

    nc.compile()
    return nc


def make_inputs(cfg, features, edge_index, W1, W2):
    c = cfg
    colidx, rowloc2, meta = prep_edges(cfg, edge_index)
    iota2d = np.broadcast_to(np.arange(256, dtype=np.float16),
                             (128, 256)).copy()
    ident = np.eye(128, dtype=np.float16)
    w1 = np.ascontiguousarray(np.asarray(W1, np.float16))
    w2 = np.ascontiguousarray(np.asarray(W2, np.float16))
    in_maps = []
    for cc in range(c.NCORE):
        xc = np.asarray(features[cc * c.NPC:(cc + 1) * c.NPC], np.float32)
        xt = np.zeros((c.IN, c.NPP), np.float16)
        xt[:, :c.NPC] = xc.T.astype(np.float16)
        in_maps.append({
            "xT": np.ascontiguousarray(xt),
            "w1": w1, "w2": w2, "ident": ident, "iotaAB": iota2d,
            "colidx": np.ascontiguousarray(colidx[cc]),
            "rowloc2": np.ascontiguousarray(rowloc2[cc]),
        })
    return in_maps, meta


_LAST_NC = None


def kernel(features, edge_index, W1, W2):
    global _LAST_NC
    cfg = CFG
    in_maps, meta = make_inputs(cfg, features, edge_index, W1, W2)
    nc = build(cfg, meta)
    _LAST_NC = nc
    res = bass_utils.run_bass_kernel_spmd(
        nc, in_maps, core_ids=list(range(cfg.NCORE)))
    return np.concatenate(
        [res.results[cc]["out"] for cc in range(cfg.NCORE)], axis=0)
